# revision 49
# baseline (speedup 1.0000x reference)
"""Trainium2 Bass kernel for nn_Encoder (gnn_message_passing).

Reference computation (per batch b, n=N nodes, H hidden):
  node_hidden[b,i,h]  = pos[b,i]*w_pos[h] + s[b,i]*w_s[h] + d_h[b,hint,i]*w_dh[h]
                        + (b_pos+b_s+b_dh)[h]
  edge_hidden[b,i,j,h] = A[b,i,j]*w_A[h] + adj[b,i,j]*w_adj[h]
                        + onehot(pi_h[b,hint,i])[j]*w_pih[h] + (b_A+b_adj+b_pih)[h]
  graph_hidden[b,h]   = gkey[b]*w_gkey[h] + b_gkey[h]
  adj_out[b,i,j]      = (eye + ((adj+adjT)>0) + ((oh+ohT)>0.5)) > 0

Sharding: pure data parallel over batch B=16 across 8 cores (2 batches/core).

Device strategy: every *_hidden is a rank-k outer product + bias, i.e. a
matmul with contraction k<=4: lhsT[k, m] = data rows (incl. a ones row for
the bias), rhs[k, h] = weight rows (incl. the summed bias row). The edge
tensor is 256 matmuls of [4,128]x[4,128] -> PSUM [j,h], copied to SBUF on
scalar/vector engines and DMA'd out. One-hot rows are built on the vector
engine (iota + is_equal vs per-partition pi values) and moved into the
edge lhsT layout with an SBUF->SBUF DMA. adjT/ohT come from PE transposes.
"""

import numpy as np

B, N, T, H = 16, 128, 8, 128
NCORES = 8
BC = B // NCORES  # batches per core

_CACHE = {}


def _patch_wait_split():
    """This walrus build accepts only ONE sync-wait command per instruction.
    Tile can attach several (final drain, matmuls waiting on multiple DMAs...).
    Post-process the serialized BIR: move excess waits onto NoOps inserted
    just before the instruction on the same engine — semantically identical
    since each engine executes its stream in order."""
    import concourse.bass as bass
    import orjson

    if getattr(bass.Bass, "_wait_split_patched", False):
        return
    orig = bass.Bass.to_json_bytes

    def to_json_bytes(self):
        m = orjson.loads(orig(self))
        ctr = [0]

        def fix(blk):
            out = []
            for ins in blk.get("instructions", []):
                si = ins.get("sync_info")
                waits = (si or {}).get("on_wait") or []
                if len(waits) > 1:
                    for w in waits[:-1]:
                        ctr[0] += 1
                        nop = {
                            "engine": ins["engine"],
                            "ins": [],
                            "outs": [],
                            "name": f"WSPLIT-{ctr[0]}",
                            "opcode": "NoOp",
                            "text_hint": "wait_split",
                            "sync_info": {"on_wait": [w], "on_update": []},
                        }
                        if ins.get("debug") is not None:
                            nop["debug"] = ins["debug"]
                        out.append(nop)
                    si["on_wait"] = waits[-1:]
                out.append(ins)
            blk["instructions"] = out
            for sub in blk.get("blocks", []):
                fix(sub)

        for f in m.get("functions", []):
            for blk in f.get("blocks", []):
                fix(blk)
        return orjson.dumps(m)

    bass.Bass.to_json_bytes = to_json_bytes
    bass.Bass._wait_split_patched = True


def build_nc():
    import concourse.bass as bass
    import concourse.tile as tile
    import concourse.mybir as mybir

    _patch_wait_split()

    f32 = mybir.dt.float32
    AL = mybir.AluOpType

    nc = bass.Bass()
    # A and adj pre-packed on host into the L layout: [b, t, r=4k+u, R, j]
    aadj_d = nc.declare_dram_parameter("Aadj_c", [BC, 4, 8, 8, N], f32, isOutput=False)
    adj_d = nc.declare_dram_parameter("adj_c", [BC, N, N], f32, isOutput=False)
    npk_d = nc.declare_dram_parameter("nodepk", [4, BC * N], f32, isOutput=False)
    wn_d = nc.declare_dram_parameter("W_node", [4, H], f32, isOutput=False)
    we_d = nc.declare_dram_parameter("W_edge", [16, 4 * H], f32, isOutput=False)
    gpk_d = nc.declare_dram_parameter("graphpk", [2, BC], f32, isOutput=False)
    wg_d = nc.declare_dram_parameter("W_graph", [2, H], f32, isOutput=False)
    pi_d = nc.declare_dram_parameter("pi3f", [BC, N], f32, isOutput=False)
    # pi values replicated into the packed L layout: [b, t, u, R*128+j]
    pip_d = nc.declare_dram_parameter("pi_rep", [BC, 4, 4, 8 * N], f32, isOutput=False)

    node_o = nc.declare_dram_parameter("node_o", [BC, N, H], f32, isOutput=True)
    edge_o = nc.declare_dram_parameter("edge_o", [BC, N, N, H], f32, isOutput=True)
    graph_o = nc.declare_dram_parameter("graph_o", [BC, H], f32, isOutput=True)
    adjo_d = nc.declare_dram_parameter("adj_o", [BC, N, N], f32, isOutput=True)

    with tile.TileContext(nc) as tc:
        with (
            tc.tile_pool(name="const", bufs=1) as cpool,
            tc.tile_pool(name="lp", bufs=2) as lpool,
            tc.tile_pool(name="small", bufs=2) as spool,
            tc.tile_pool(name="eo", bufs=6) as epool,
            tc.tile_pool(name="pse", bufs=5, space="PSUM") as pse,
            tc.tile_pool(name="pst", bufs=1, space="PSUM") as pst,
            tc.tile_pool(name="psn", bufs=1, space="PSUM") as psn,
            tc.tile_pool(name="dram", bufs=2, space="DRAM") as dpool,
        ):
            # ---- constants ----
            picol = cpool.tile([N, BC], f32)  # first: feeds the one-hot path
            nc.sync.dma_start(out=picol, in_=pi_d.rearrange("b i -> i b"))
            iota_rep = cpool.tile([N, N], f32)  # value j on every partition
            nc.gpsimd.iota(
                iota_rep,
                pattern=[[1, N]],
                base=0,
                channel_multiplier=0,
                allow_small_or_imprecise_dtypes=True,
            )
            iota_col = cpool.tile([N, 1], f32)  # value i per partition
            nc.gpsimd.iota(
                iota_col,
                pattern=[[1, 1]],
                base=0,
                channel_multiplier=1,
                allow_small_or_imprecise_dtypes=True,
            )
            eye = cpool.tile([N, N], f32)
            nc.vector.tensor_scalar(
                out=eye, in0=iota_rep, scalar1=iota_col, scalar2=None, op0=AL.is_equal
            )
            iota16 = cpool.tile([N, 8 * N], f32)  # j-mod-128 on each partition
            nc.gpsimd.iota(
                iota16,
                pattern=[[0, 8], [1, N]],
                base=0,
                channel_multiplier=0,
                allow_small_or_imprecise_dtypes=True,
            )

            # Block-diagonal edge weights W16[4u+k, u*128+h] = W_edge[k,h],
            # replicated at partition groups 0/32/64/96 (both matmul operands
            # must share base_partition == tile_position[0]). One K=16 matmul
            # computes 4 node-tiles; 4 row-groups run concurrently on the PE.
            we_sb = cpool.tile([N, 4 * H], f32)
            nc.gpsimd.memset(we_sb[:], 0.0)
            for t in range(4):
                nc.sync.dma_start(out=we_sb[32 * t : 32 * t + 16, :], in_=we_d[:])
            wn_sb = cpool.tile([4, H], f32)
            nc.sync.dma_start(out=wn_sb, in_=wn_d[:])
            wg_sb = cpool.tile([2, H], f32)
            nc.sync.dma_start(out=wg_sb, in_=wg_d[:])
            npk_sb = cpool.tile([4, BC * N], f32)
            nc.sync.dma_start(out=npk_sb, in_=npk_d[:])
            gpk_sb = cpool.tile([2, BC], f32)
            nc.sync.dma_start(out=gpk_sb, in_=gpk_d[:])

            # ---- node hidden: per batch [4,128]x[4,128] -> [i,h] ----
            nps = psn.tile([N, BC * H], f32)
            for b in range(BC):
                nc.tensor.matmul(
                    nps[:, b * H : (b + 1) * H],
                    npk_sb[:, b * N : (b + 1) * N],
                    wn_sb,
                    start=True,
                    stop=True,
                )
            nsb = spool.tile([N, BC * H], f32)
            nc.scalar.copy(out=nsb, in_=nps)
            # SBUF side stays partition-major; permute the DRAM side instead
            nc.scalar.dma_start(
                out=node_o.rearrange("b i h -> i b h"), in_=nsb
            )

            # ---- graph hidden: [2,BC]x[2,H] -> [b,h] ----
            gps = psn.tile([BC, H], f32)
            nc.tensor.matmul(gps, gpk_sb, wg_sb, start=True, stop=True)
            gsb = spool.tile([BC, H], f32)
            nc.vector.tensor_copy(out=gsb, in_=gps)
            nc.sync.dma_start(out=graph_o[:], in_=gsb)

            # ---- per-batch: edge hidden + adjacency ----
            for b in range(BC):
                # Loads for batch 0 go on the idle-at-start SP HWDGE ring;
                # batch 1 loads use GpSimd SWDGE so they don't queue behind
                # batch 0's edge stores (FIFO per ring) and don't eat the
                # ~0.6us/issue SP sequencer budget.
                ld = nc.sync if b == 0 else nc.gpsimd
                # edge lhsT rows: A | adj | ones | onehot. Node i = R*16 +
                # t*4 + u lives at partition 32*t + 4*u + k, free R*128 + j
                # (k-major row order -> contiguous (t,k) partition blocks).
                L = lpool.tile([N, 8 * N], f32)
                # ones rows get 1.0 from the memset; pad rows 32t+16..31 too
                # (their weights are 0)
                nc.gpsimd.memset(L[:], 1.0)
                # row blocks: k=0 onehot (at the 32-aligned group base, since
                # engine outputs need 32-aligned partitions), k=1 A, k=2 adj,
                # k=3 ones (from the memset)
                Lvf = L.rearrange("(t r) (R j) -> t r R j", r=32, j=N)
                for t in range(4):
                    ld.dma_start(out=Lvf[t, 4:12], in_=aadj_d[b, t])

                # one-hot rows straight into L: compare host-replicated pi
                # values against the j-mod-128 iota (no DRAM roundtrip).
                # pip rows live at the 32-aligned group bases like L's.
                pip_sb = lpool.tile([N, 8 * N], f32)
                pipv = pip_sb.rearrange("(t r) f -> t r f", r=32)
                for t in range(4):
                    ld.dma_start(out=pipv[t, 0:4], in_=pip_d[b, t])
                for t in range(4):
                    nc.vector.tensor_tensor(
                        out=Lvf[t, 0:4],
                        in0=pipv[t, 0:4],
                        in1=iota16[32 * t : 32 * t + 4, :],
                        op=AL.is_equal,
                    )

                # edge hidden: 8 rounds x 16 i-tiles. Each round: 4 K=16
                # block-diag matmuls (one per PE row-group, 4 tiles each).
                for R in range(8):
                    eo = epool.tile([N, 16 * H], f32)
                    for t in range(4):
                        ps = pse.tile([N, 4 * H], f32)
                        nc.tensor.matmul(
                            ps,
                            L[32 * t : 32 * t + 16, R * N : (R + 1) * N],
                            we_sb[32 * t : 32 * t + 16, :],
                            start=True,
                            stop=True,
                            tile_position=(32 * t, 0),
                        )
                        if t % 2 == 0:
                            nc.scalar.copy(
                                out=eo[:, t * 4 * H : (t + 1) * 4 * H], in_=ps
                            )
                        else:
                            nc.vector.tensor_copy(
                                out=eo[:, t * 4 * H : (t + 1) * 4 * H], in_=ps
                            )
                    # split the 1MB store across the two HWDGE rings (SP+ACT)
                    # so the 16 SDMA engines always have two queues to drain
                    nc.sync.dma_start(
                        out=edge_o[b, R * 16 : R * 16 + 8].rearrange(
                            "i j h -> j i h"
                        ),
                        in_=eo[:, : 8 * H],
                    )
                    nc.scalar.dma_start(
                        out=edge_o[b, R * 16 + 8 : R * 16 + 16].rearrange(
                            "i j h -> j i h"
                        ),
                        in_=eo[:, 8 * H :],
                    )

                # adjacency output (off the edge critical path)
                oh = spool.tile([N, N], f32)  # oh[i,j] = (pi3[i]==j)
                nc.vector.tensor_scalar(
                    out=oh,
                    in0=iota_rep,
                    scalar1=picol[:, b : b + 1],
                    scalar2=None,
                    op0=AL.is_equal,
                )
                adjsb = spool.tile([N, N], f32)
                ld.dma_start(out=adjsb, in_=adj_d[b])
                tpa = pst.tile([N, N], f32, tag="tp")
                nc.tensor.transpose(tpa, adjsb, eye)
                adjT = spool.tile([N, N], f32)
                nc.scalar.copy(out=adjT, in_=tpa)
                tpo = pst.tile([N, N], f32, tag="tp")
                nc.tensor.transpose(tpo, oh, eye)
                ohT = spool.tile([N, N], f32)
                nc.scalar.copy(out=ohT, in_=tpo)

                t1 = spool.tile([N, N], f32)
                nc.vector.tensor_tensor(out=t1, in0=adjsb, in1=adjT, op=AL.add)
                nc.vector.tensor_scalar(
                    out=t1, in0=t1, scalar1=0.0, scalar2=None, op0=AL.is_gt
                )
                t2 = spool.tile([N, N], f32)
                nc.vector.tensor_tensor(out=t2, in0=oh, in1=ohT, op=AL.add)
                nc.vector.tensor_scalar(
                    out=t2, in0=t2, scalar1=0.5, scalar2=None, op0=AL.is_gt
                )
                nc.vector.tensor_tensor(out=t1, in0=t1, in1=t2, op=AL.add)
                nc.vector.tensor_tensor(out=t1, in0=t1, in1=eye, op=AL.add)
                nc.vector.tensor_scalar(
                    out=t1, in0=t1, scalar1=0.5, scalar2=None, op0=AL.is_gt
                )
                nc.scalar.dma_start(out=adjo_d[b], in_=t1)
    return nc


def _prep_in_maps(inputs):
    f = lambda x: np.ascontiguousarray(np.asarray(x), dtype=np.float32)
    hint = int(np.asarray(inputs["hint_step"]))
    pos, s = f(inputs["pos"]), f(inputs["s"])
    A, adj = f(inputs["A"]), f(inputs["adj"])
    dh3 = f(np.asarray(inputs["d_h"])[:, hint])
    pi3 = f(np.asarray(inputs["pi_h"])[:, hint])
    gkey = f(inputs["gkey"])
    w = {k: f(inputs[k]) for k in inputs if k.startswith(("w_", "b_"))}

    bsum_e = w["b_A"] + w["b_adj"] + w["b_pih"]
    bnode = w["b_pos"] + w["b_s"] + w["b_dh"]
    # row-block order matches the device L layout: onehot | A | adj | ones
    we4 = np.stack([w["w_pih"], w["w_A"], w["w_adj"], bsum_e])  # [4, H]
    W_edge = np.zeros((16, 4 * H), np.float32)  # block-diag K=16, k-major rows
    for k in range(4):
        for u in range(4):
            W_edge[4 * k + u, u * H : (u + 1) * H] = we4[k]
    W_node = np.ascontiguousarray(np.stack([w["w_pos"], w["w_s"], w["w_dh"], bnode]))
    W_graph = np.ascontiguousarray(np.stack([w["w_gkey"], w["b_gkey"]]))
    ones_bn = np.ones(BC * N, np.float32)
    ones_b = np.ones(BC, np.float32)

    # pack A/adj into the device L layout: [b, t, r=4k+u, R, j]
    Ap = A.reshape(B, 8, 4, 4, N).transpose(0, 2, 3, 1, 4)  # [b, t, u, R, j]
    adjp = adj.reshape(B, 8, 4, 4, N).transpose(0, 2, 3, 1, 4)
    Aadj = np.empty((B, 4, 8, 8, N), np.float32)
    Aadj[:, :, 0:4] = Ap
    Aadj[:, :, 4:8] = adjp
    # pi values replicated along j in the packed layout: [b, t, u, R*128+j]
    pi_rep = np.ascontiguousarray(
        np.broadcast_to(
            pi3.reshape(B, 8, 4, 4).transpose(0, 2, 3, 1)[..., None],  # b,t,u,R,1
            (B, 4, 4, 8, N),
        ).reshape(B, 4, 4, 8 * N)
    )

    in_maps = []
    for c in range(NCORES):
        sl = slice(BC * c, BC * (c + 1))
        nodepk = np.ascontiguousarray(
            np.stack([pos[sl].ravel(), s[sl].ravel(), dh3[sl].ravel(), ones_bn])
        )
        graphpk = np.ascontiguousarray(np.stack([gkey[sl], ones_b]))
        in_maps.append(
            dict(
                Aadj_c=np.ascontiguousarray(Aadj[sl]),
                adj_c=np.ascontiguousarray(adj[sl]),
                nodepk=nodepk,
                W_node=W_node,
                W_edge=W_edge,
                graphpk=graphpk,
                W_graph=W_graph,
                pi3f=np.ascontiguousarray(pi3[sl]),
                pi_rep=np.ascontiguousarray(pi_rep[sl]),
            )
        )
    return in_maps


def kernel(**inputs):
    from concourse.bass_utils import run_bass_kernel_spmd

    if "nc" not in _CACHE:
        _CACHE["nc"] = build_nc()
    nc = _CACHE["nc"]

    in_maps = _prep_in_maps(inputs)
    res = run_bass_kernel_spmd(nc, in_maps, core_ids=list(range(NCORES)))
    node = np.concatenate([r["node_o"] for r in res.results], axis=0)
    edge = np.concatenate([r["edge_o"] for r in res.results], axis=0)
    graph = np.concatenate([r["graph_o"] for r in res.results], axis=0)
    adjo = np.concatenate([r["adj_o"] for r in res.results], axis=0)
    return node, edge, graph, adjo


# revision 52
# speedup vs baseline: 1.0425x; 1.0425x over previous
"""Trainium2 Bass kernel for nn_Encoder (gnn_message_passing).

Reference computation (per batch b, n=N nodes, H hidden):
  node_hidden[b,i,h]  = pos[b,i]*w_pos[h] + s[b,i]*w_s[h] + d_h[b,hint,i]*w_dh[h]
                        + (b_pos+b_s+b_dh)[h]
  edge_hidden[b,i,j,h] = A[b,i,j]*w_A[h] + adj[b,i,j]*w_adj[h]
                        + onehot(pi_h[b,hint,i])[j]*w_pih[h] + (b_A+b_adj+b_pih)[h]
  graph_hidden[b,h]   = gkey[b]*w_gkey[h] + b_gkey[h]
  adj_out[b,i,j]      = (eye + ((adj+adjT)>0) + ((oh+ohT)>0.5)) > 0

Sharding: pure data parallel over batch B=16 across 8 cores (2 batches/core).

Device strategy: every *_hidden is a rank-k outer product + bias, i.e. a
matmul with contraction k<=4: lhsT[k, m] = data rows (incl. a ones row for
the bias), rhs[k, h] = weight rows (incl. the summed bias row). The edge
tensor is 256 matmuls of [4,128]x[4,128] -> PSUM [j,h], copied to SBUF on
scalar/vector engines and DMA'd out. One-hot rows are built on the vector
engine (iota + is_equal vs per-partition pi values) and moved into the
edge lhsT layout with an SBUF->SBUF DMA. adjT/ohT come from PE transposes.
"""

import numpy as np

B, N, T, H = 16, 128, 8, 128
NCORES = 8
BC = B // NCORES  # batches per core

_CACHE = {}


def _patch_wait_split():
    """This walrus build accepts only ONE sync-wait command per instruction.
    Tile can attach several (final drain, matmuls waiting on multiple DMAs...).
    Post-process the serialized BIR: move excess waits onto NoOps inserted
    just before the instruction on the same engine — semantically identical
    since each engine executes its stream in order."""
    import concourse.bass as bass
    import orjson

    if getattr(bass.Bass, "_wait_split_patched", False):
        return
    orig = bass.Bass.to_json_bytes

    def to_json_bytes(self):
        m = orjson.loads(orig(self))
        ctr = [0]

        def fix(blk):
            out = []
            for ins in blk.get("instructions", []):
                si = ins.get("sync_info")
                waits = (si or {}).get("on_wait") or []
                if len(waits) > 1:
                    for w in waits[:-1]:
                        ctr[0] += 1
                        nop = {
                            "engine": ins["engine"],
                            "ins": [],
                            "outs": [],
                            "name": f"WSPLIT-{ctr[0]}",
                            "opcode": "NoOp",
                            "text_hint": "wait_split",
                            "sync_info": {"on_wait": [w], "on_update": []},
                        }
                        if ins.get("debug") is not None:
                            nop["debug"] = ins["debug"]
                        out.append(nop)
                    si["on_wait"] = waits[-1:]
                out.append(ins)
            blk["instructions"] = out
            for sub in blk.get("blocks", []):
                fix(sub)

        for f in m.get("functions", []):
            for blk in f.get("blocks", []):
                fix(blk)
        return orjson.dumps(m)

    bass.Bass.to_json_bytes = to_json_bytes
    bass.Bass._wait_split_patched = True


def build_nc():
    import concourse.bass as bass
    import concourse.tile as tile
    import concourse.mybir as mybir

    _patch_wait_split()

    f32 = mybir.dt.float32
    AL = mybir.AluOpType

    nc = bass.Bass()
    # A/adj/ones pre-packed on host into the L layout: [b, t, r-4, R, j]
    aadj_d = nc.declare_dram_parameter("Aadj_c", [BC, 4, 12, 8, N], f32, isOutput=False)
    adj_d = nc.declare_dram_parameter("adj_c", [BC, N, N], f32, isOutput=False)
    # misc pack: rows 0:4 = [W_node | nodepk], rows 32:34 = [W_graph | graphpk]
    misc_d = nc.declare_dram_parameter("miscpk", [64, H + BC * N], f32, isOutput=False)
    we_d = nc.declare_dram_parameter("W_edge", [N, 4 * H], f32, isOutput=False)
    pi_d = nc.declare_dram_parameter("pi3f", [BC, N], f32, isOutput=False)
    # pi values replicated into the packed L layout: [b, t, u, R*128+j]
    pip_d = nc.declare_dram_parameter("pi_rep", [BC, 4, 4, 8 * N], f32, isOutput=False)

    node_o = nc.declare_dram_parameter("node_o", [BC, N, H], f32, isOutput=True)
    edge_o = nc.declare_dram_parameter("edge_o", [BC, N, N, H], f32, isOutput=True)
    graph_o = nc.declare_dram_parameter("graph_o", [BC, H], f32, isOutput=True)
    adjo_d = nc.declare_dram_parameter("adj_o", [BC, N, N], f32, isOutput=True)

    with tile.TileContext(nc) as tc:
        with (
            tc.tile_pool(name="const", bufs=1) as cpool,
            tc.tile_pool(name="lp", bufs=2) as lpool,
            tc.tile_pool(name="small", bufs=2) as spool,
            tc.tile_pool(name="eo", bufs=6) as epool,
            tc.tile_pool(name="pse", bufs=5, space="PSUM") as pse,
            tc.tile_pool(name="pst", bufs=1, space="PSUM") as pst,
            tc.tile_pool(name="psn", bufs=1, space="PSUM") as psn,
            tc.tile_pool(name="dram", bufs=2, space="DRAM") as dpool,
        ):
            # ---- constants ----
            picol = cpool.tile([N, BC], f32)  # first: feeds the one-hot path
            nc.sync.dma_start(out=picol, in_=pi_d.rearrange("b i -> i b"))
            iota_rep = cpool.tile([N, N], f32)  # value j on every partition
            nc.gpsimd.iota(
                iota_rep,
                pattern=[[1, N]],
                base=0,
                channel_multiplier=0,
                allow_small_or_imprecise_dtypes=True,
            )
            iota_col = cpool.tile([N, 1], f32)  # value i per partition
            nc.gpsimd.iota(
                iota_col,
                pattern=[[1, 1]],
                base=0,
                channel_multiplier=1,
                allow_small_or_imprecise_dtypes=True,
            )
            eye = cpool.tile([N, N], f32)
            nc.vector.tensor_scalar(
                out=eye, in0=iota_rep, scalar1=iota_col, scalar2=None, op0=AL.is_equal
            )
            iota16 = cpool.tile([N, 8 * N], f32)  # j-mod-128 on each partition
            nc.gpsimd.iota(
                iota16,
                pattern=[[0, 8], [1, N]],
                base=0,
                channel_multiplier=0,
                allow_small_or_imprecise_dtypes=True,
            )

            # Block-diagonal edge weights W16[4u+k, u*128+h] = W_edge[k,h],
            # replicated at partition groups 0/32/64/96 (both matmul operands
            # must share base_partition == tile_position[0]). One K=16 matmul
            # computes 4 node-tiles; 4 row-groups run concurrently on the PE.
            we_sb = cpool.tile([N, 4 * H], f32)
            nc.sync.dma_start(out=we_sb, in_=we_d[:])
            misc_sb = cpool.tile([64, H + BC * N], f32)
            nc.sync.dma_start(out=misc_sb, in_=misc_d[:])
            wn_sb = misc_sb[0:4, 0:H]
            npk_sb = misc_sb[0:4, H : H + BC * N]
            wg_sb = misc_sb[32:34, 0:H]
            gpk_sb = misc_sb[32:34, H : H + BC]

            # ---- node hidden: per batch [4,128]x[4,128] -> [i,h] ----
            nps = psn.tile([N, BC * H], f32)
            for b in range(BC):
                nc.tensor.matmul(
                    nps[:, b * H : (b + 1) * H],
                    npk_sb[:, b * N : (b + 1) * N],
                    wn_sb,
                    start=True,
                    stop=True,
                )
            nsb = spool.tile([N, BC * H], f32)
            nc.scalar.copy(out=nsb, in_=nps)
            # SBUF side stays partition-major; permute the DRAM side instead
            nc.scalar.dma_start(
                out=node_o.rearrange("b i h -> i b h"), in_=nsb
            )

            # ---- graph hidden: [2,BC]x[2,H] -> [b,h] ----
            gps = psn.tile([BC, H], f32)
            nc.tensor.matmul(gps, gpk_sb, wg_sb, start=True, stop=True)
            gsb = spool.tile([BC, H], f32)
            nc.vector.tensor_copy(out=gsb, in_=gps)
            nc.sync.dma_start(out=graph_o[:], in_=gsb)

            # ---- per-batch: edge hidden + adjacency ----
            for b in range(BC):
                # Loads for batch 0 go on the idle-at-start SP HWDGE ring;
                # batch 1 loads use GpSimd SWDGE so they don't queue behind
                # batch 0's edge stores (FIFO per ring) and don't eat the
                # ~0.6us/issue SP sequencer budget.
                ld = nc.sync if b == 0 else nc.gpsimd
                # edge lhsT rows: A | adj | ones | onehot. Node i = R*16 +
                # t*4 + u lives at partition 32*t + 4*u + k, free R*128 + j
                # (k-major row order -> contiguous (t,k) partition blocks).
                L = lpool.tile([N, 8 * N], f32)
                # pad rows 32t+16..31: value irrelevant (weights 0) but the
                # sim's rounded-up weight read needs them initialized
                nc.gpsimd.memset(L[:], 0.0)
                # row blocks: k=0 onehot (at the 32-aligned group base, since
                # engine outputs need 32-aligned partitions), k=1 A, k=2 adj,
                # k=3 ones (from the memset)
                Lvf = L.rearrange("(t r) (R j) -> t r R j", r=32, j=N)
                for t in range(4):
                    ld.dma_start(out=Lvf[t, 4:16], in_=aadj_d[b, t])

                # one-hot rows straight into L: compare host-replicated pi
                # values against the j-mod-128 iota (no DRAM roundtrip).
                # pip rows live at the 32-aligned group bases like L's.
                pip_sb = lpool.tile([N, 8 * N], f32)
                pipv = pip_sb.rearrange("(t r) f -> t r f", r=32)
                for t in range(4):
                    ld.dma_start(out=pipv[t, 0:4], in_=pip_d[b, t])
                for t in range(4):
                    nc.vector.tensor_tensor(
                        out=Lvf[t, 0:4],
                        in0=pipv[t, 0:4],
                        in1=iota16[32 * t : 32 * t + 4, :],
                        op=AL.is_equal,
                    )

                # edge hidden: 8 rounds x 16 i-tiles. Each round: 4 K=16
                # block-diag matmuls (one per PE row-group, 4 tiles each).
                for R in range(8):
                    eo = epool.tile([N, 16 * H], f32)
                    for t in range(4):
                        ps = pse.tile([N, 4 * H], f32)
                        nc.tensor.matmul(
                            ps,
                            L[32 * t : 32 * t + 16, R * N : (R + 1) * N],
                            we_sb[32 * t : 32 * t + 16, :],
                            start=True,
                            stop=True,
                            tile_position=(32 * t, 0),
                        )
                        if t % 2 == 0:
                            nc.scalar.copy(
                                out=eo[:, t * 4 * H : (t + 1) * 4 * H], in_=ps
                            )
                        else:
                            nc.vector.tensor_copy(
                                out=eo[:, t * 4 * H : (t + 1) * 4 * H], in_=ps
                            )
                    # split the 1MB store across the two HWDGE rings (SP+ACT)
                    # so the 16 SDMA engines always have two queues to drain
                    nc.sync.dma_start(
                        out=edge_o[b, R * 16 : R * 16 + 8].rearrange(
                            "i j h -> j i h"
                        ),
                        in_=eo[:, : 8 * H],
                    )
                    nc.scalar.dma_start(
                        out=edge_o[b, R * 16 + 8 : R * 16 + 16].rearrange(
                            "i j h -> j i h"
                        ),
                        in_=eo[:, 8 * H :],
                    )

                # adjacency output (off the edge critical path)
                oh = spool.tile([N, N], f32)  # oh[i,j] = (pi3[i]==j)
                nc.vector.tensor_scalar(
                    out=oh,
                    in0=iota_rep,
                    scalar1=picol[:, b : b + 1],
                    scalar2=None,
                    op0=AL.is_equal,
                )
                adjsb = spool.tile([N, N], f32)
                (nc.scalar if b == 0 else nc.gpsimd).dma_start(
                    out=adjsb, in_=adj_d[b]
                )
                tpa = pst.tile([N, N], f32, tag="tp")
                nc.tensor.transpose(tpa, adjsb, eye)
                adjT = spool.tile([N, N], f32)
                nc.scalar.copy(out=adjT, in_=tpa)
                tpo = pst.tile([N, N], f32, tag="tp")
                nc.tensor.transpose(tpo, oh, eye)
                ohT = spool.tile([N, N], f32)
                nc.scalar.copy(out=ohT, in_=tpo)

                t1 = spool.tile([N, N], f32)
                nc.vector.tensor_tensor(out=t1, in0=adjsb, in1=adjT, op=AL.add)
                nc.vector.tensor_scalar(
                    out=t1, in0=t1, scalar1=0.0, scalar2=None, op0=AL.is_gt
                )
                t2 = spool.tile([N, N], f32)
                nc.vector.tensor_tensor(out=t2, in0=oh, in1=ohT, op=AL.add)
                nc.vector.tensor_scalar(
                    out=t2, in0=t2, scalar1=0.5, scalar2=None, op0=AL.is_gt
                )
                nc.vector.tensor_tensor(out=t1, in0=t1, in1=t2, op=AL.add)
                nc.vector.tensor_tensor(out=t1, in0=t1, in1=eye, op=AL.add)
                nc.vector.tensor_scalar(
                    out=t1, in0=t1, scalar1=0.5, scalar2=None, op0=AL.is_gt
                )
                nc.scalar.dma_start(out=adjo_d[b], in_=t1)
    return nc


def _prep_in_maps(inputs):
    f = lambda x: np.ascontiguousarray(np.asarray(x), dtype=np.float32)
    hint = int(np.asarray(inputs["hint_step"]))
    pos, s = f(inputs["pos"]), f(inputs["s"])
    A, adj = f(inputs["A"]), f(inputs["adj"])
    dh3 = f(np.asarray(inputs["d_h"])[:, hint])
    pi3 = f(np.asarray(inputs["pi_h"])[:, hint])
    gkey = f(inputs["gkey"])
    w = {k: f(inputs[k]) for k in inputs if k.startswith(("w_", "b_"))}

    bsum_e = w["b_A"] + w["b_adj"] + w["b_pih"]
    bnode = w["b_pos"] + w["b_s"] + w["b_dh"]
    # row-block order matches the device L layout: onehot | A | adj | ones
    we4 = np.stack([w["w_pih"], w["w_A"], w["w_adj"], bsum_e])  # [4, H]
    # full [128, 512] layout: block-diag K=16 replicated at partitions
    # 32t..32t+16, zeros in the pad rows -> a single plain DMA on device
    W_edge = np.zeros((N, 4 * H), np.float32)
    for t in range(4):
        for k in range(4):
            for u in range(4):
                W_edge[32 * t + 4 * k + u, u * H : (u + 1) * H] = we4[k]

    # pack A/adj/ones into the device L layout rows 4:16: [b, t, r-4, R, j]
    Ap = A.reshape(B, 8, 4, 4, N).transpose(0, 2, 3, 1, 4)  # [b, t, u, R, j]
    adjp = adj.reshape(B, 8, 4, 4, N).transpose(0, 2, 3, 1, 4)
    Aadj = np.empty((B, 4, 12, 8, N), np.float32)
    Aadj[:, :, 0:4] = Ap
    Aadj[:, :, 4:8] = adjp
    Aadj[:, :, 8:12] = 1.0
    # pi values replicated along j in the packed layout: [b, t, u, R*128+j]
    pi_rep = np.ascontiguousarray(
        np.broadcast_to(
            pi3.reshape(B, 8, 4, 4).transpose(0, 2, 3, 1)[..., None],  # b,t,u,R,1
            (B, 4, 4, 8, N),
        ).reshape(B, 4, 4, 8 * N)
    )

    in_maps = []
    for c in range(NCORES):
        sl = slice(BC * c, BC * (c + 1))
        # misc pack: rows 0:4 = [W_node | nodepk], rows 32:34 = [W_graph | graphpk]
        misc = np.zeros((64, H + BC * N), np.float32)
        misc[0:4, 0:H] = np.stack([w["w_pos"], w["w_s"], w["w_dh"], bnode])
        misc[0:4, H:] = np.stack(
            [pos[sl].ravel(), s[sl].ravel(), dh3[sl].ravel(), np.ones(BC * N)]
        )
        misc[32:34, 0:H] = np.stack([w["w_gkey"], w["b_gkey"]])
        misc[32:34, H : H + BC] = np.stack([gkey[sl], np.ones(BC)])
        in_maps.append(
            dict(
                Aadj_c=np.ascontiguousarray(Aadj[sl]),
                adj_c=np.ascontiguousarray(adj[sl]),
                miscpk=misc,
                W_edge=W_edge,
                pi3f=np.ascontiguousarray(pi3[sl]),
                pi_rep=np.ascontiguousarray(pi_rep[sl]),
            )
        )
    return in_maps


def kernel(**inputs):
    from concourse.bass_utils import run_bass_kernel_spmd

    if "nc" not in _CACHE:
        _CACHE["nc"] = build_nc()
    nc = _CACHE["nc"]

    in_maps = _prep_in_maps(inputs)
    res = run_bass_kernel_spmd(nc, in_maps, core_ids=list(range(NCORES)))
    node = np.concatenate([r["node_o"] for r in res.results], axis=0)
    edge = np.concatenate([r["edge_o"] for r in res.results], axis=0)
    graph = np.concatenate([r["graph_o"] for r in res.results], axis=0)
    adjo = np.concatenate([r["adj_o"] for r in res.results], axis=0)
    return node, edge, graph, adjo
